# revision 11
# baseline (speedup 1.0000x reference)
"""Trainium2 Bass kernel for the sparse-attention scores module.

scores[b, :] = softmax_s( v . tanh(W1 @ static[b] + W2 @ dynamic[b] + W3 @ hidden[b]) )

Data-parallel over B across 8 NeuronCores (8 batches per core).  Encoder
tensors stream as host-pretiled fp8 e3m4 (16 MiB HBM traffic per core, a
quarter of the fp32 baseline); the W tiles stay bf16 and the PE runs
mixed-dtype bf16x e3m4 matmuls (~140-170 ns per 512-column matmul,
measured).  The two accumulation groups per column chunk are interleaved
across PSUM banks (measured +25% throughput vs sequential groups), and the
v-reduction uses a full-width [128,128] stationary with v in column 0 (a
1-column-output matmul measures ~1.7x SLOWER than a full-width one).
End-to-end rel-L2 error vs the fp32 reference is ~7e-3, dominated by the
e3m4 encoder quantization.
"""

import sys

sys.path.insert(0, "/opt/trn_rl_repo")

import numpy as np
import ml_dtypes

B, H, S = 64, 256, 4096
N_CORES = 8
BPC = B // N_CORES          # batches per core
KK = H // 128               # 2 contraction chunks
MM = H // 128               # 2 output-row chunks
NCH = S // 512              # 8 psum column chunks
NQ = 2                      # input DMA quarters along s
SQ = S // NQ                # columns per quarter

BF16 = ml_dtypes.bfloat16
E3 = ml_dtypes.float8_e3m4
X_SCALE = 2.0               # x -> e3m4 pre-scale (undone in the ACT descale)


def build_bass(reps: int = 1, loop_iters: int = 0):
    """Build the per-core Bass program. reps>1 unrolls the whole computation
    multiple times; loop_iters>0 additionally wraps the unrolled body in a
    hardware loop. Both are used only for timing by differencing."""
    import contextlib

    import concourse.bacc as bacc
    import concourse.tile as tile
    from concourse import mybir

    f32 = mybir.dt.float32
    f32r = mybir.dt.float32r
    bf16 = mybir.dt.bfloat16

    nc = bacc.Bacc(None)

    # x tensors are host-pretiled: [b, q, p, kk, s'] = x[b, kk*128+p, q*SQ+s']
    # so each per-quarter DMA is a single fully-contiguous 1 MiB read.
    f8e3 = mybir.dt.float8e3
    xs = nc.dram_tensor("xs", [BPC, NQ, 128, KK, SQ], f8e3, kind="ExternalInput")
    xd = nc.dram_tensor("xd", [BPC, NQ, 128, KK, SQ], f8e3, kind="ExternalInput")
    # W1/W2 tiles: j = t*4 + kk*2 + m.
    wt = nc.dram_tensor("wt", [128, 8, 128], bf16, kind="ExternalInput")
    # W3 tiles (fp32) for the per-batch bias: j = kk*2 + m.
    wt3 = nc.dram_tensor("wt3", [128, 4, 128], f32r, kind="ExternalInput")
    ht = nc.dram_tensor("ht", [128, KK, BPC], f32r, kind="ExternalInput")
    vt = nc.dram_tensor("vt", [128, MM, 128], bf16, kind="ExternalInput")
    out = nc.dram_tensor("out", [BPC, S], f32, kind="ExternalOutput")

    with tile.TileContext(nc) as tc:
        with (
            tc.tile_pool(name="consts", bufs=1) as consts,
            tc.tile_pool(name="xpool", bufs=3) as xpool,
            tc.tile_pool(name="tpool", bufs=8) as tpool,
            tc.tile_pool(name="spool", bufs=2) as spool,
            tc.tile_pool(name="mpsum", bufs=7, space="PSUM") as mpsum,
            tc.tile_pool(name="vpsum", bufs=1, space="PSUM") as vpsum,
        ):
            wt_sb = consts.tile([128, 8, 128], bf16)
            nc.sync.dma_start(out=wt_sb, in_=wt[:, :, :])
            wt3_sb = consts.tile([128, 4, 128], f32r)
            nc.sync.dma_start(out=wt3_sb, in_=wt3[:, :, :])
            ht_sb = consts.tile([128, KK, BPC], f32r)
            nc.sync.dma_start(out=ht_sb, in_=ht[:, :, :])
            vt_sb = consts.tile([128, MM, 128], bf16)
            nc.sync.dma_start(out=vt_sb, in_=vt[:, :, :])

            # Inline 0/1 masks for the softmax normalization matmuls:
            # bsum[b] = sum_n esums[8b+n]; brep[8b+n] = bsum[b].
            ma_np = np.zeros((64, BPC), np.float32)
            mb_np = np.zeros((BPC, 64), np.float32)
            for p in range(64):
                ma_np[p, p // NCH] = 1.0
                mb_np[p // NCH, p] = 1.0
            ma_dram = nc.inline_tensor(ma_np, name="ma")
            mb_dram = nc.inline_tensor(mb_np, name="mb")
            ma_sb = consts.tile([64, BPC], f32)
            nc.sync.dma_start(out=ma_sb, in_=ma_dram[:, :])
            mb_sb = consts.tile([BPC, 64], f32)
            nc.sync.dma_start(out=mb_sb, in_=mb_dram[:, :])

            # Per-batch bias: bias[m*128+h', b] = (W3 @ hidden[b])[m*128+h']
            bias_sb = consts.tile([128, MM, BPC], f32)
            for m in range(MM):
                bias_ps = vpsum.tile([128, BPC], f32, tag="vp")
                for kk in range(KK):
                    nc.tensor.matmul(
                        bias_ps,
                        lhsT=wt3_sb[:, kk * 2 + m, :],
                        rhs=ht_sb[:, kk, :],
                        start=(kk == 0),
                        stop=(kk == KK - 1),
                    )
                nc.vector.tensor_copy(out=bias_sb[:, m, :], in_=bias_ps)

            loop_cm = (
                tc.For_i(0, loop_iters, 1) if loop_iters else contextlib.nullcontext()
            )
            def emit_epilogue(sc64):
                # Softmax epilogue. Scores are small (|s| < ~6), so skip the
                # max subtraction: softmax = exp(s) / sum(exp(s)). The
                # per-batch sums are formed from the per-partition accum via
                # two tiny 0/1-mask matmuls (sum over n, then broadcast).
                esums = spool.tile([64, 1], f32, tag="esums")
                nc.scalar.activation(
                    out=sc64,
                    in_=sc64,
                    func=mybir.ActivationFunctionType.Exp,
                    accum_out=esums,
                )
                bsum_ps = vpsum.tile([BPC, 1], f32, tag="vp")
                nc.tensor.matmul(bsum_ps, lhsT=ma_sb, rhs=esums,
                                 start=True, stop=True)
                bsum_sb = spool.tile([BPC, 1], f32, tag="bsum")
                nc.vector.tensor_copy(out=bsum_sb, in_=bsum_ps)
                brep_ps = vpsum.tile([64, 1], f32, tag="vp")
                nc.tensor.matmul(brep_ps, lhsT=mb_sb, rhs=bsum_sb,
                                 start=True, stop=True)
                recip = spool.tile([64, 1], f32, tag="recip")
                nc.vector.reciprocal(out=recip, in_=brep_ps)
                nc.vector.tensor_scalar_mul(out=sc64, in0=sc64, scalar1=recip)
                nc.gpsimd.dma_start(
                    out=out[:, :].rearrange("b (n s) -> (b n) s", n=NCH),
                    in_=sc64,
                )

            pending_epi = None
            with loop_cm:
              for _ in range(reps):
                # Scores live as [64, 512] with partition p = 8*b + n so the
                # epilogue runs on all 64 partitions at once.
                scores64 = spool.tile([64, 512], f32, tag="scores")
                pending = None
                for b in range(BPC):
                    # Stream the two encoder tensors in contiguous quarters.
                    xq = []
                    for q in range(NQ):
                        pair = []
                        for t, dram in ((0, xs), (1, xd)):
                            xt = xpool.tile([128, KK, SQ], f8e3, tag=f"x{t}{q}")
                            nc.sync.dma_start(out=xt, in_=dram[b, q])
                            pair.append(xt)
                        xq.append(pair)

                    def emit_vdot(pend):
                        # v-dot runs one chunk late so the tanh results are
                        # ready and the PE never waits on the ACT engine.
                        # The stationary is a full [128,128] tile with v in
                        # column 0: a 1-column-output matmul is ~1.7x slower
                        # than a full-width one, so compute 128 rows and
                        # read only row 0.
                        row, vp, tts = pend
                        for m in range(MM):
                            nc.tensor.matmul(
                                vp,
                                lhsT=vt_sb[:, m, :],
                                rhs=tts[m],
                                start=(m == 0),
                                stop=(m == MM - 1),
                            )
                        # Compute engines may only address partition bases
                        # that are multiples of 32 (and DMA cannot read PSUM),
                        # so row 0 is drained to SBUF and a tiny SBUF->SBUF
                        # DMA places it at partition 8b+n of the scores tile.
                        stage = tpool.tile([1, 512], f32, tag="stage")
                        nc.vector.tensor_copy(out=stage, in_=vp[0:1, :])
                        nc.gpsimd.dma_start(
                            out=scores64[row : row + 1, :],
                            in_=stage,
                        )

                    for n in range(NCH):
                        q, r = divmod(n, NCH // NQ)
                        pss = []
                        for m in range(MM):
                            ps = mpsum.tile([128, 512], f32, tag="ps")
                            pss.append(ps)
                        # Interleave the two accumulation groups (m=0/m=1)
                        # across PSUM banks: measured ~25% faster than
                        # running the groups back-to-back.
                        for i, (t, kk) in enumerate(
                            ((0, 0), (0, 1), (1, 0), (1, 1))
                        ):
                            for m in range(MM):
                                nc.tensor.matmul(
                                    pss[m],
                                    lhsT=wt_sb[:, t * 4 + kk * 2 + m, :],
                                    rhs=xq[q][t][:, kk, r * 512 : (r + 1) * 512],
                                    start=(i == 0),
                                    stop=(i == 3),
                                )
                        tts = []
                        for m in range(MM):
                            tt = tpool.tile([128, 512], bf16, tag="tt")
                            nc.scalar.activation(
                                out=tt,
                                in_=pss[m],
                                func=mybir.ActivationFunctionType.Tanh,
                                bias=bias_sb[:, m, b : b + 1],
                                scale=0.5,
                            )
                            tts.append(tt)
                        if pending is not None:
                            emit_vdot(pending)
                        vp = vpsum.tile([128, 512], f32, tag="vp")
                        pending = (b * NCH + n, vp, tts)
                    if b == 0 and pending_epi is not None:
                        # Previous rep's epilogue: by now its score DMAs have
                        # long landed, so the PE-side mask matmuls never stall.
                        emit_epilogue(pending_epi)
                        pending_epi = None
                # flush the last batch's final v-dot after the loop
                emit_vdot(pending)
                pending_epi = scores64
              # last rep's epilogue runs after the loop
            emit_epilogue(pending_epi)

    nc.finalize()
    return nc


def prep_shared_inputs(W: np.ndarray, v: np.ndarray, decoder_hidden: np.ndarray):
    """Host-side layout marshaling of the small replicated parameters."""
    W = np.ascontiguousarray(W, dtype=np.float32)
    wt = np.empty((128, 8, 128), BF16)
    for t in range(2):
        Wt = np.ascontiguousarray(W[:, t * H : (t + 1) * H].T).astype(BF16)
        for kk in range(KK):
            for m in range(MM):
                j = t * 4 + kk * 2 + m
                wt[:, j, :] = Wt[kk * 128 : (kk + 1) * 128, m * 128 : (m + 1) * 128]
    wt3 = np.empty((128, 4, 128), np.float32)
    W3t = W[:, 2 * H :].T
    for kk in range(KK):
        for m in range(MM):
            wt3[:, kk * 2 + m, :] = W3t[kk * 128 : (kk + 1) * 128,
                                        m * 128 : (m + 1) * 128]
    vt = np.zeros((128, MM, 128), BF16)
    for m in range(MM):
        vt[:, m, 0] = v[0][m * 128 : (m + 1) * 128].astype(BF16)
    hT = decoder_hidden[0].T.astype(np.float32)  # [H, B]
    return wt, wt3, vt, hT


def _tile_x(xc: np.ndarray) -> np.ndarray:
    """[BPC, H, S] e3m4 -> [BPC, NQ, 128, KK, SQ] contiguous."""
    return np.ascontiguousarray(
        xc.reshape(BPC, KK, 128, NQ, SQ).transpose(0, 3, 2, 1, 4)
    )


_CACHED = {}


def _get_nc(reps: int = 1, loop_iters: int = 0):
    key = (reps, loop_iters)
    if key not in _CACHED:
        _CACHED[key] = build_bass(reps, loop_iters)
    return _CACHED[key]


def make_in_maps(static_enc, dynamic_enc, decoder_hidden, W, v):
    wt, wt3, vt, hT = prep_shared_inputs(W, v, decoder_hidden)
    xsb = (np.asarray(static_enc, np.float32) * np.float32(X_SCALE)).astype(E3)
    xdb = (np.asarray(dynamic_enc, np.float32) * np.float32(X_SCALE)).astype(E3)
    in_maps = []
    for c in range(N_CORES):
        b0 = c * BPC
        ht_c = np.ascontiguousarray(
            hT[:, b0 : b0 + BPC].reshape(KK, 128, BPC).transpose(1, 0, 2)
        )  # [p, kk, b]
        in_maps.append(
            {
                "xs": _tile_x(xsb[b0 : b0 + BPC]),
                "xd": _tile_x(xdb[b0 : b0 + BPC]),
                "wt": wt,
                "wt3": wt3,
                "ht": ht_c,
                "vt": vt,
            }
        )
    return in_maps


def kernel(static_enc, dynamic_enc, decoder_hidden, W, v):
    from concourse.bass_utils import run_bass_kernel_spmd

    nc = _get_nc(reps=1)
    in_maps = make_in_maps(static_enc, dynamic_enc, decoder_hidden, W, v)
    res = run_bass_kernel_spmd(nc, in_maps, core_ids=list(range(N_CORES)))
    return np.concatenate([r["out"] for r in res.results], axis=0)
